# revision 38
# baseline (speedup 1.0000x reference)
"""Trainium2 Bass kernel for nn_CenterAgent (scatter_memory).

Self-contained: takes FULL inputs (B=256), shards batch across 8 NeuronCores
(pure data parallel, 32 samples/core), runs one Bass/Tile program per core via
run_bass_kernel_spmd, gathers the full [256, 24] output.

v4 structure (per core).  Every DMA costs ~0.7us of its issuing queue
(completion-synchronous HWDGE), so the design minimizes DMA count:
  - consts packed into 4 group tensors (one DMA each + c_fw1p in 2 chunks).
  - features loaded 2 samples per DMA (16 DMAs) into f32 staging, cast to
    fp8 on the DVE (gpsimd's software DGE is slow and stays nearly empty).
  - stage0 (512->128 channel contraction at 7x7, fp8 DR) -> staging sES
    [98, 1280] (3 scalar copies) -> ONE row-interleaved DMA per sample into
    Hs[s] [98, 640]: U-block row r = (pos r//2, r%2 ? odd-tap : even-tap),
    block 4 = tap8 on even rows, zero coefficients on odd rows.
  - conv1 = im2col (pairs (dj0,dj2) stride-2 over a one-DMA col30) + 5 bf16
    U matmuls (K=98), N=392 per half via 4-dim moving APs (no pad columns).
  - conv2/3/4: 5 fp8-DR matmuls per (sample, half): overlapping-AP pairs
    (0,3),(1,4),(2,5) stride-30, (6,8) stride-2, single 7 -- no shifted
    copies.  Sample packing across shrinking channel dims via staging +
    partition-shift DMAs on the HWDGE queues.  One merged act per sample.
  - One PSUM ring (2-bank "big" x3 + 1-bank "tr" x2) for all phases.
  - Tail: fp8 PE transposes, fc1 fp8 DR, fc2 f32r.
"""

from contextlib import ExitStack

import ml_dtypes
import numpy as np

import concourse.bass as bass
import concourse.tile as tile
from concourse import bacc, mybir
from concourse.bass_utils import run_bass_kernel_spmd

NCORES = 8
B = 256
BL = B // NCORES  # 32 samples per core
SC = 8            # samples per pipeline chunk
F32 = mybir.dt.float32
F32R = mybir.dt.float32r
F16 = mybir.dt.float16
BF16 = mybir.dt.bfloat16
F8 = mybir.dt.float8e4
I16 = mybir.dt.int16
U8 = mybir.dt.uint8
ALU = mybir.AluOpType
ACTF = mybir.ActivationFunctionType
DR = mybir.MatmulPerfMode.DoubleRow

EVEN = [0, 2, 4, 6]
ODD = [1, 3, 5, 7]
# conv tap-pair slots: (t0, t1, moving-pair stride in the [32,30] slot)
PAIRS = [(0, 3, 30), (1, 4, 30), (2, 5, 30), (6, 8, 2), (7, None, 30)]

f8 = ml_dtypes.float8_e4m3

# ----------------------------------------------------------- packed const maps
# fp8 pack: per-partition byte layout [128, NF8]
_F8_LAYOUT = {
    "w1fe": (0, [4, 512]),      # stage0 even-tap weights
    "w1fo": (2048, [4, 512]),
    "w1f8": (4096, [4, 128]),
    "w1ic": (4608, [2, 2, 128]),  # im2col (only partitions 0-11 meaningful)
    "w2p": (5120, [5, 2, 64]),
    "w3p": (5760, [5, 2, 64]),
    "w4p": (6400, [5, 2, 64]),
    "ident8": (7040, [64]),
}
NF8 = 7104
# f32 pack: per-partition float layout [128, NF32]
_F32_LAYOUT = {
    "b1": (0, [1]),
    "b2": (1, [1]),
    "b3": (2, [1]),
    "b4": (3, [1]),
    "k27": (4, [27]),
    "identf": (31, [32]),       # rows 0-31 = eye(32)
}
NF32 = 64  # padded for DMA alignment
# f32r pack (PE-rounded floats): fw2/fb1/fb2
_FR_LAYOUT = {
    "fw2": (0, [2, 24]),
    "fb1": (48, [256]),         # row 0 only
    "fb2": (304, [24]),         # row 0 only
}
NFR = 336


def _bilinear_A():
    A = np.zeros((28, 7), np.float32)
    for i in range(28):
        t = (i + 0.5) / 4 - 0.5
        p0 = int(np.floor(t))
        w = t - p0
        A[i, min(max(p0, 0), 6)] += 1 - w
        A[i, min(max(p0 + 1, 0), 6)] += w
    return A


def _shifted_A(d):
    A = _bilinear_A()
    S = np.zeros_like(A)
    for i in range(28):
        src = i + d - 1
        if 0 <= src < 28:
            S[i] = A[src]
    return S


def _utap(t):
    di, dj = divmod(t, 3)
    return np.einsum(
        "ip,jq->pqij", _shifted_A(di), _shifted_A(dj)
    ).reshape(49, 784).astype(np.float32)


def _wpair(wt_list):
    K, M = wt_list[0].shape
    out = np.zeros((K, 5, 2, M), np.float32)
    for si, (t0, t1, _) in enumerate(PAIRS):
        out[:, si, 0, :] = wt_list[t0]
        if t1 is not None:
            out[:, si, 1, :] = wt_list[t1]
    return out


def _build_consts(w1, b1, w2, b2, w3, b3, w4, b4, fw1, fb1, fw2, fb2):
    w1 = np.asarray(w1, np.float32)
    w1f = w1[:, 3:515]  # [128o, 512c, 3, 3]
    pf8 = np.zeros((128, NF8), np.float32)

    def put8(name, arr):
        off, shape = _F8_LAYOUT[name]
        n = int(np.prod(shape))
        pf8[:arr.shape[0], off:off + n] = arr.reshape(arr.shape[0], n)

    w1fe = np.zeros((128, 4, 512), np.float32)
    w1fo = np.zeros((128, 4, 512), np.float32)
    w1f8c = np.zeros((128, 4, 128), np.float32)
    for kb in range(4):
        blk = w1f[:, kb * 128:(kb + 1) * 128]
        for ti, t in enumerate(EVEN):
            di, dj = divmod(t, 3)
            w1fe[:, kb, ti * 128:(ti + 1) * 128] = blk[:, :, di, dj].T
        for ti, t in enumerate(ODD):
            di, dj = divmod(t, 3)
            w1fo[:, kb, ti * 128:(ti + 1) * 128] = blk[:, :, di, dj].T
        w1f8c[:, kb, :] = blk[:, :, 2, 2].T
    put8("w1fe", w1fe)
    put8("w1fo", w1fo)
    put8("w1f8", w1f8c)

    chmap = [0, 1, 2, 515]
    w1ic = np.zeros((12, 2, 2, 128), np.float32)
    for di in range(3):
        for ch in range(4):
            w1ic[di * 4 + ch, 0, 0] = w1[:, chmap[ch], di, 0]
            w1ic[di * 4 + ch, 0, 1] = w1[:, chmap[ch], di, 2]
            w1ic[di * 4 + ch, 1, 0] = w1[:, chmap[ch], di, 1]
    put8("w1ic", w1ic)

    w2 = np.asarray(w2, np.float32)
    put8("w2p", _wpair([w2[:, :, t // 3, t % 3].T for t in range(9)]))
    w3 = np.asarray(w3, np.float32)
    w3t = []
    for t in range(9):
        wt = np.zeros((128, 64), np.float32)
        blk = w3[:, :, t // 3, t % 3].T
        wt[0:64, 0:32] = blk
        wt[64:128, 32:64] = blk
        w3t.append(wt)
    put8("w3p", _wpair(w3t))
    w4 = np.asarray(w4, np.float32)
    w4t = []
    for t in range(9):
        wt = np.zeros((128, 64), np.float32)
        blk = w4[:, :, t // 3, t % 3].T
        for bi in range(4):
            wt[32 * bi:32 * bi + 32, 16 * bi:16 * bi + 16] = blk
        w4t.append(wt)
    put8("w4p", _wpair(w4t))

    ident8 = np.zeros((128, 64), np.float32)
    ident8[0:64] = np.eye(64)
    ident8[64:128] = np.eye(64)
    put8("ident8", ident8)

    pf32 = np.zeros((128, NF32), np.float32)

    def put32(name, arr):
        off, shape = _F32_LAYOUT[name]
        n = int(np.prod(shape))
        pf32[:arr.shape[0], off:off + n] = arr.reshape(arr.shape[0], n)

    put32("b1", np.asarray(b1, np.float32).reshape(128, 1))
    put32("b2", np.tile(np.asarray(b2, np.float32), 2).reshape(128, 1))
    put32("b3", np.tile(np.asarray(b3, np.float32), 4).reshape(128, 1))
    put32("b4", np.tile(np.asarray(b4, np.float32), 8).reshape(128, 1))
    put32("k27", np.broadcast_to(np.arange(1, 28, dtype=np.float32),
                                 (128, 27)).copy())
    put32("identf", np.eye(32, dtype=np.float32))

    pfr = np.zeros((128, NFR), np.float32)

    def putr(name, arr):
        off, shape = _FR_LAYOUT[name]
        n = int(np.prod(shape))
        pfr[:arr.shape[0], off:off + n] = arr.reshape(arr.shape[0], n)

    fw2 = np.asarray(fw2, np.float32)
    fw2c = np.zeros((128, 2, 24), np.float32)
    fw2c[:, 0] = fw2[:, 0:128].T
    fw2c[:, 1] = fw2[:, 128:256].T
    putr("fw2", fw2c)
    putr("fb1", np.asarray(fb1, np.float32).reshape(1, 256))
    putr("fb2", np.asarray(fb2, np.float32).reshape(1, 24))

    # c_uall [98, 5, 784] bf16 with row r = (pos r//2, parity r%2):
    # parity 0 -> EVEN[t] (block 4: tap8), parity 1 -> ODD[t] (block 4: 0)
    uall = np.zeros((98, 5, 784), np.float32)
    for t in range(4):
        uall[0::2, t] = _utap(EVEN[t])
        uall[1::2, t] = _utap(ODD[t])
    uall[0::2, 4] = _utap(8)

    # fc1: [112, 7, 8chpair, 2, 256] fp8
    f3 = np.asarray(fw1, np.float32).reshape(256, 16, 784)
    c_fw1p = np.zeros((112, 7, 8, 2, 256), np.float32)
    for c in range(7):
        blk = f3[:, :, 112 * c:112 * (c + 1)].transpose(2, 1, 0)
        c_fw1p[:, c] = blk.reshape(112, 8, 2, 256)

    tri2 = np.zeros((4, 25, 100), np.float32)
    for b in range(4):
        for i in range(25):
            tri2[b, i, 25 * b + i + 1:] = 1.0
    c_tri2 = np.broadcast_to(tri2[:, None], (4, 32, 25, 100)).reshape(128, 25, 100)

    return {
        "cst_f8": pf8.astype(f8),
        "cst_f32": pf32,
        "cst_fr": pfr,
        "c_uall": uall.astype(ml_dtypes.bfloat16),
        "c_fw1p": c_fw1p.astype(f8),
        "c_tri2": np.ascontiguousarray(c_tri2).astype(np.float16),
    }


_CONST_SPECS = {
    "cst_f8": ([128, NF8], F8),
    "cst_f32": ([128, NF32], F32),
    "cst_fr": ([128, NFR], F32R),
    "c_uall": ([98, 5, 784], BF16),
    "c_fw1p": ([112, 7, 8, 2, 256], F8),
    "c_tri2": ([128, 25, 100], F16),
}


def _win4(buf_ap, base, pstride, n_rows=14):
    """4-dim moving AP: [K, 2(kpair @pstride), n_rows(@30), 28(@1)]."""
    ap_list = [tuple(e) for e in buf_ap.ap]
    return bass.AP(buf_ap.tensor, buf_ap.offset + base,
                   [ap_list[0], (pstride, 2), (30, n_rows), (1, 28)])


def _wb(h, di, dj):
    # out (r, c) tap (di, dj) reads slot row 14h + r + di + 1, col c + dj
    return (1 + 14 * h + di) * 30 + dj


# ------------------------------------------------------------------ device IR


def build_nc():
    nc = bacc.Bacc("TRN2", target_bir_lowering=False, debug=False)
    image = nc.dram_tensor("image", [BL, 3, 28, 28], F32R, kind="ExternalInput").ap()
    features = nc.dram_tensor("features", [BL, 512, 7, 7], F32R, kind="ExternalInput").ap()
    centers = nc.dram_tensor("centers", [BL, 100, 4], F32, kind="ExternalInput").ap()
    cst = {
        name: nc.dram_tensor(name, shape, dt, kind="ExternalInput").ap()
        for name, (shape, dt) in _CONST_SPECS.items()
    }
    out_d = nc.dram_tensor("out", [BL, 24], F32, kind="ExternalOutput").ap()
    scratch = nc.dram_tensor("scratch", [BL, 4, 30, 30], F8, kind="Internal").ap()

    with tile.TileContext(nc) as tc, ExitStack() as ctx:
        cp = ctx.enter_context(tc.tile_pool(name="consts", bufs=1))
        pp = ctx.enter_context(tc.tile_pool(name="persist", bufs=1))
        hp = ctx.enter_context(tc.tile_pool(name="hbuf", bufs=1))
        fwp = ctx.enter_context(tc.tile_pool(name="fw1", bufs=1))
        sp = ctx.enter_context(tc.tile_pool(name="scat", bufs=1))
        sgp = ctx.enter_context(tc.tile_pool(name="s0stg", bufs=4))
        colp = ctx.enter_context(tc.tile_pool(name="col", bufs=8))
        stgp = ctx.enter_context(tc.tile_pool(name="stg", bufs=2))
        smp = ctx.enter_context(tc.tile_pool(name="small", bufs=2))
        psp = ctx.enter_context(tc.tile_pool(name="psum", bufs=3, space="PSUM"))

        def big(nm):
            return psp.tile([128, 2, 512], F32, tag="big", name=nm)

        # const tiles + views
        cf8 = cp.tile([128, NF8], F8, tag="cf8")
        cf32 = cp.tile([128, NF32], F32, tag="cf32")
        cfr = cp.tile([128, NFR], F32R, tag="cfr")
        c_uall = cp.tile([98, 5, 784], BF16, tag="c_uall")
        c_tri2 = cp.tile([128, 25, 100], F16, tag="c_tri2")
        nc.sync.dma_start(out=cf8[:], in_=cst["cst_f8"])
        nc.scalar.dma_start(out=cf32[:], in_=cst["cst_f32"])
        nc.scalar.dma_start(out=cfr[:], in_=cst["cst_fr"])
        nc.scalar.dma_start(out=c_uall[:], in_=cst["c_uall"])
        nc.scalar.dma_start(out=c_tri2[:], in_=cst["c_tri2"])

        def _view(tile_ap, rows, off, shape):
            n = int(np.prod(shape))
            v = tile_ap[0:rows, off:off + n]
            if len(shape) == 2:
                v = v.rearrange("p (a b) -> p a b", a=shape[0])
            elif len(shape) == 3:
                v = v.rearrange("p (a b c) -> p a b c", a=shape[0], b=shape[1])
            return v

        def v8(name, rows=128):
            off, shape = _F8_LAYOUT[name]
            return _view(cf8, rows, off, shape)

        def v32(name, rows=128):
            off, shape = _F32_LAYOUT[name]
            return _view(cf32, rows, off, shape)

        def vfr(name, rows=128):
            off, shape = _FR_LAYOUT[name]
            return _view(cfr, rows, off, shape)

        fbuf = pp.tile([128, 16, 4, 112], F8, tag="fbuf")
        x1buf = pp.tile([128, SC, 32, 30], F8, tag="x1buf")
        x2buf = pp.tile([128, 4, 32, 30], F8, tag="x2buf")
        x3buf = pp.tile([128, 2, 32, 30], F8, tag="x3buf")
        x4t = pp.tile([128, 2, 392], F8, tag="x4t")
        x4T = pp.tile([112, 7, 8, 2, 32], F8, tag="x4T")
        Hs = [hp.tile([98, 640], BF16, tag=f"H{s}", name=f"Hs{s}")
              for s in range(BL)]
        c_fw1_t = fwp.tile([112, 7, 8, 2, 256], F8, tag="c_fw1", name="c_fw1_t")

        # ---------------- feature staging: 2 samples per DMA, DVE casts
        # fstg [128, 2smp, 4kb, 49]: (smp, kb) merge into one stride-6272 dim
        def emit_fstg(g):
            fs = sgp.tile([128, 2, 4, 49], F32R, tag="fstg", name=f"fstg{g}")
            q = nc.scalar if g % 2 == 0 else nc.gpsimd
            q.dma_start(
                out=fs[:],
                in_=bass.AP(features.tensor,
                            features.offset + 2 * g * 512 * 49,
                            [(49, 128), (128 * 49, 8), (1, 49)]))
            return fs

        fstg = {g: emit_fstg(g) for g in range(4)}

        # ---------------- scatter inputs
        cen4 = sp.tile([128, 25, 4], F32, tag="cen4")
        nc.sync.dma_start(
            out=cen4[:],
            in_=bass.AP(centers.tensor, centers.offset,
                        [(100, 4), (400, 32), (4, 25), (1, 4)]))

        ipad = sp.tile([96, 30, 30], F8, tag="ipad")
        nc.gpsimd.memset(ipad[:].bitcast(U8), 0)
        nc.gpsimd.dma_start(out=ipad[:, 1:29, 1:29],
                            in_=image.rearrange("s c h w -> (s c) h w"))
        nc.sync.dma_start(out=scratch[:, 0:3], in_=ipad[:])

        # ---------------- DVE: feature casts interleaved with scatter compute
        def cast_group(g):
            nc.vector.tensor_copy(
                fbuf[:, g, :, 0:98].rearrange("p k (h x) -> p h k x", h=2),
                fstg.pop(g)[:].bitcast(F32))

        for g in range(3):
            cast_group(g)

        ge = sp.tile([128, 25, 27], F32, tag="ge")

        def floor28(dst, coord_ap, name):
            v = sp.tile([128, 25], F32, tag=name, name=name)
            nc.vector.tensor_scalar_mul(v[:], coord_ap, 28.0)
            nc.vector.tensor_tensor(
                ge[:],
                v[:].unsqueeze(2).broadcast_to([128, 25, 27]),
                v32("k27").unsqueeze(1).broadcast_to([128, 25, 27]),
                ALU.is_ge,
            )
            nc.vector.tensor_reduce(dst[:], ge[:], mybir.AxisListType.X,
                                    ALU.add)

        xp = sp.tile([128, 25], F32, tag="xp")
        floor28(xp, cen4[:, :, 0], "xs")
        yp = sp.tile([128, 25], F32, tag="yp")
        floor28(yp, cen4[:, :, 1], "ys")
        flat = sp.tile([128, 25], F32, tag="flat")
        nc.vector.scalar_tensor_tensor(flat[:], yp[:], 30.0, xp[:],
                                       ALU.mult, ALU.add)
        nc.vector.tensor_scalar_add(flat[:], flat[:], 31.0)
        flat16 = sp.tile([128, 25], F16, tag="flat16")
        nc.vector.tensor_copy(flat16[:], flat[:])

        cast_group(3)

        flat_s = sp.tile([32, 100], F16, tag="flat_s")
        for b in range(4):
            nc.scalar.dma_start(out=flat_s[:, 25 * b:25 * b + 25],
                                in_=flat16[32 * b:32 * b + 32, :])
        flatAll = sp.tile([128, 100], F16, tag="flatAll")
        for b in range(4):
            nc.scalar.dma_start(out=flatAll[32 * b:32 * b + 32, :],
                                in_=flat_s[:])

        D = sp.tile([128, 25, 100], F16, tag="D")
        nc.vector.tensor_tensor(
            D[:],
            flat16[:].unsqueeze(2).broadcast_to([128, 25, 100]),
            flatAll[:].unsqueeze(1).broadcast_to([128, 25, 100]),
            ALU.is_equal)
        E = sp.tile([128, 25, 100], F16, tag="E")
        nc.vector.tensor_mul(E[:], D[:], c_tri2[:])
        later = sp.tile([128, 25], F16, tag="later")
        nc.vector.tensor_reduce(later[:], E[:], mybir.AxisListType.X,
                                ALU.max)
        lateri = sp.tile([128, 25], U8, tag="lateri")
        nc.vector.tensor_copy(lateri[:], later[:])
        neg1 = sp.tile([128, 25], F32, tag="neg1")
        nc.vector.memset(neg1[:], -1.0)
        idxf = sp.tile([128, 25], F32, tag="idxf")
        nc.vector.select(idxf[:], lateri[:], neg1[:], flat[:])
        idx16p = sp.tile([128, 25], I16, tag="idx16p")
        nc.vector.tensor_copy(idx16p[:], idxf[:])
        conf16p = sp.tile([128, 25], F16, tag="conf16p")
        nc.vector.tensor_copy(conf16p[:], cen4[:, :, 3])

        ones32 = cp.tile([1, 32], F32R, tag="ones32")
        nc.vector.memset(ones32[:].bitcast(F32), 1.0)
        # zero the conv pad borders
        for buf, ns in ((x1buf, SC), (x2buf, 4), (x3buf, 2)):
            bf = buf[:].rearrange("p s a b -> p (s a b)")
            for base, cnt in ((0, 60), (900, 60)):
                nc.vector.memset(
                    bass.AP(bf.tensor, bf.offset + base,
                            [tuple(bf.ap[0]), (960, ns), (1, cnt)]).bitcast(U8), 0)
            nc.vector.memset(
                bass.AP(bf.tensor, bf.offset + 60,
                        [tuple(bf.ap[0]), (960, ns), (30, 28), (29, 2)]).bitcast(U8), 0)

        # gpsimd: scatter tail
        idx_s = sp.tile([32, 100], I16, tag="idx_s")
        conf_s = sp.tile([32, 100], F16, tag="conf_s")
        for b in range(4):
            nc.gpsimd.dma_start(out=idx_s[:, 25 * b:25 * b + 25],
                                in_=idx16p[32 * b:32 * b + 32, :])
            nc.gpsimd.dma_start(out=conf_s[:, 25 * b:25 * b + 25],
                                in_=conf16p[32 * b:32 * b + 32, :])
        cmap16 = sp.tile([32, 900], F16, tag="cmap16")
        nc.gpsimd.local_scatter(cmap16[:], conf_s[:], idx_s[:],
                                channels=32, num_elems=900, num_idxs=100)
        cmap8 = sp.tile([32, 900], F8, tag="cmap8")
        nc.vector.tensor_copy(cmap8[:], cmap16[:])
        nc.gpsimd.dma_start(
            out=scratch[:, 3].rearrange("s a b -> s (a b)"),
            in_=cmap8[:])

        # col30 loads: ONE DMA per sample, prefetched a chunk ahead
        col_tiles = {}

        def emit_col30(s, q):
            col30 = colp.tile([12, 30, 30], F8, tag="col30", name=f"col30_{s}")
            nc.vector.memset(col30[:, 29, :].bitcast(U8), 0)
            q.dma_start(
                out=col30[:, 1:29, :],
                in_=bass.AP(scratch.tensor, scratch.offset + s * 3600,
                            [(30, 3), (900, 4), (30, 28), (1, 30)]))
            col_tiles[s] = col30

        for s in range(SC):
            emit_col30(s, nc.sync)

        # ---------------------------------------------- stage0 (4 groups per
        # chunk; groups 0-3 up front, 4..15 interleaved into chunk bodies)
        def emit_stage0_group(g):
            psE = big("psE")
            psO = big("psO")
            for ki in range(2):
                lhs = fbuf[:, g, 2 * ki:2 * ki + 2, 0:98]
                nc.tensor.matmul(
                    psE[0:98, 0, :], lhs,
                    v8("w1fe")[:, 2 * ki:2 * ki + 2, :],
                    start=(ki == 0), stop=(ki == 1), perf_mode=DR)
                nc.tensor.matmul(
                    psO[0:98, 0, :], lhs,
                    v8("w1fo")[:, 2 * ki:2 * ki + 2, :],
                    start=(ki == 0), stop=(ki == 1), perf_mode=DR)
                nc.tensor.matmul(
                    psE[0:98, 1, 0:128], lhs,
                    v8("w1f8")[:, 2 * ki:2 * ki + 2, :],
                    start=(ki == 0), stop=(ki == 1), perf_mode=DR)
            # staging sES [98, 1280] bf16 = [even 512 | tap8 128 | odd 512
            #                                | tap8-dup 128]
            sES = sgp.tile([98, 1280], BF16, tag="sES", name="sES")
            nc.scalar.copy(
                sES[:, 0:640],
                psE[0:98, :, :].rearrange("p a b -> p (a b)")[:, 0:640])
            nc.scalar.copy(sES[:, 640:1152], psO[0:98, 0, :])
            nc.scalar.copy(sES[:, 1152:1280], psE[0:98, 1, 0:128])
            for half in range(2):
                h = Hs[2 * g + half]
                src = sES[49 * half:49 * half + 49, :]
                nc.sync.dma_start(
                    out=h[:],
                    in_=bass.AP(src.tensor, src.offset,
                                [tuple(src.ap[0]), (640, 2), (1, 640)]))

        for g in range(4):
            emit_stage0_group(g)

        # ---------------------------------------------- conv pipeline
        def conv_dr(ps_out, wconst, xbuf, slot, h):
            xa = xbuf[:, slot].rearrange("p a b -> p (a b)")
            for si_, (t0, _, strd) in enumerate(PAIRS):
                di, dj = divmod(t0, 3)
                nc.tensor.matmul(
                    ps_out, wconst[:, si_, :, :],
                    _win4(xa, _wb(h, di, dj), strd),
                    start=(si_ == 0), stop=(si_ == 4), perf_mode=DR)

        def act2(dst_interior, ps, bias, n=64):
            nc.scalar.activation(
                dst_interior.rearrange("p (a b) c -> p a b c", a=2),
                ps[0:n, :, 0:392].rearrange("p a (b c) -> p a b c", b=14),
                ACTF.Relu, bias=bias, scale=1.0)

        for ci in range(4):
            for si in range(SC):
                s = ci * SC + si
                col30 = col_tiles.pop(s)
                cf = col30[:].rearrange("p a b -> p (a b)")
                ps1 = big("ps1")
                for h in range(2):
                    o_ap = ps1[:, h, 0:392]
                    nc.tensor.matmul(
                        o_ap, v8("w1ic", rows=12)[:, 0, :, :],
                        _win4(cf, 30 * (1 + 14 * h), 2),
                        start=True, stop=False, perf_mode=DR)
                    nc.tensor.matmul(
                        o_ap, v8("w1ic", rows=12)[:, 1, :, :],
                        _win4(cf, 30 * (1 + 14 * h) + 1, 2),
                        start=False, stop=False, perf_mode=DR)
                    for t in range(5):
                        nc.tensor.matmul(
                            o_ap,
                            Hs[s][:, 128 * t:128 * (t + 1)],
                            c_uall[:, t, 392 * h:392 * (h + 1)],
                            start=False, stop=(t == 4))
                act2(x1buf[:, si, 2:30, 1:29], ps1, v32("b1"), n=128)

            for p in range(4):
                for half in range(2):
                    s2 = 2 * p + half
                    ps2 = big("ps2")
                    for h in range(2):
                        conv_dr(ps2[0:64, h, 0:392], v8("w2p"), x1buf, s2, h)
                    if half == 0:
                        act2(x2buf[0:64, p, 2:30, 1:29], ps2, v32("b2")[0:64])
                    else:
                        stg2 = stgp.tile([64, 2, 392], F8, tag="stg2", name="stg2")
                        nc.scalar.activation(
                            stg2[:].rearrange("p a (b c) -> p a b c", b=14),
                            ps2[0:64, :, 0:392].rearrange(
                                "p a (b c) -> p a b c", b=14),
                            ACTF.Relu, bias=v32("b2")[0:64], scale=1.0)
                        nc.sync.dma_start(
                            out=x2buf[64:128, p, 2:30, 1:29].rearrange(
                                "p (a b) c -> p a b c", a=2),
                            in_=stg2[:].rearrange("p a (b c) -> p a b c", b=14))

            # stage0 for chunk ci+1 rides this chunk's queue slack
            if ci < 3:
                for g in range(4 * ci + 4, 4 * ci + 8):
                    fstg[g] = emit_fstg(g)
                    cast_group(g)
                    emit_stage0_group(g)

            for q_ in range(2):
                for half in range(2):
                    pp_ = 2 * q_ + half
                    ps3 = big("ps3")
                    for h in range(2):
                        conv_dr(ps3[0:64, h, 0:392], v8("w3p"), x2buf, pp_, h)
                    if half == 0:
                        act2(x3buf[0:64, q_, 2:30, 1:29], ps3, v32("b3")[0:64])
                    else:
                        stg3 = stgp.tile([64, 2, 392], F8, tag="stg3", name="stg3")
                        nc.scalar.activation(
                            stg3[:].rearrange("p a (b c) -> p a b c", b=14),
                            ps3[0:64, :, 0:392].rearrange(
                                "p a (b c) -> p a b c", b=14),
                            ACTF.Relu, bias=v32("b3")[0:64], scale=1.0)
                        nc.scalar.dma_start(
                            out=x3buf[64:128, q_, 2:30, 1:29].rearrange(
                                "p (a b) c -> p a b c", a=2),
                            in_=stg3[:].rearrange("p a (b c) -> p a b c", b=14))

            for g4 in range(2):
                ps4 = big("ps4")
                for h in range(2):
                    conv_dr(ps4[0:64, h, 0:392], v8("w4p"), x3buf, g4, h)
                if g4 == 0:
                    nc.scalar.activation(
                        x4t[0:64, :, :].rearrange("p a (b c) -> p a b c", b=14),
                        ps4[0:64, :, 0:392].rearrange("p a (b c) -> p a b c", b=14),
                        ACTF.Relu, bias=v32("b4")[0:64], scale=1.0)
                else:
                    stg4 = stgp.tile([64, 2, 392], F8, tag="stg4", name="stg4")
                    nc.scalar.activation(
                        stg4[:].rearrange("p a (b c) -> p a b c", b=14),
                        ps4[0:64, :, 0:392].rearrange("p a (b c) -> p a b c", b=14),
                        ACTF.Relu, bias=v32("b4")[0:64], scale=1.0)
                    nc.sync.dma_start(out=x4t[64:128, :, :], in_=stg4[:])

            for h2 in range(2):
                g = ci * 2 + h2
                for c in range(7):
                    tr = psp.tile([112, 64, 2], F8, tag="tr", name="tr", bufs=2)
                    nc.tensor.transpose(
                        tr[:, :, 0],
                        x4t[64 * h2:64 * h2 + 64, :, :].rearrange(
                            "p a b -> p (a b)")[:, 112 * c:112 * (c + 1)],
                        v8("ident8")[64 * h2:64 * h2 + 64, 0:64],
                    )
                    nc.vector.tensor_copy(
                        x4T[:, c, :, :, 4 * g:4 * g + 4],
                        tr[:, :, 0].rearrange("p (s cp c2) -> p cp c2 s",
                                              s=4, cp=8))

            # pin the big fc1-weight transfers behind this chunk's transposes
            # (a dep-free DMA would be hoisted into phase A by the scheduler)
            # and run them on the otherwise-idle gpsimd queue
            if ci == 1:
                nc.vector.tensor_copy(c_fw1_t[0:1, 0, 0, 0, 0:1],
                                      x4T[0:1, 0, 0, 0, 0:1])
                nc.gpsimd.dma_start(out=c_fw1_t[:, 0:4], in_=cst["c_fw1p"][:, 0:4])
            elif ci == 2:
                nc.vector.tensor_copy(c_fw1_t[0:1, 4, 0, 0, 0:1],
                                      x4T[0:1, 0, 0, 0, 8:9])
                nc.gpsimd.dma_start(out=c_fw1_t[:, 4:7], in_=cst["c_fw1p"][:, 4:7])

            if ci < 3:
                for sn in range((ci + 1) * SC, (ci + 2) * SC):
                    emit_col30(sn, nc.sync if sn % 2 == 0 else nc.scalar)

        # ------------------------------------------------ fc1 / fc2
        psF = big("psF")
        nc.tensor.matmul(psF[0:32, 0, 0:256], ones32[:],
                         vfr("fb1", rows=1),
                         start=True, stop=False)
        for c in range(7):
            for cp_ in range(8):
                nc.tensor.matmul(
                    psF[0:32, 0, 0:256],
                    x4T[:, c, cp_, :, :],
                    c_fw1_t[:, c, cp_, :, :],
                    start=False, stop=(c == 6 and cp_ == 7), perf_mode=DR)
        x5 = smp.tile([32, 256], F32, tag="x5")
        nc.scalar.activation(x5[:], psF[0:32, 0, 0:256], ACTF.Relu)

        x5T = smp.tile([128, 2, 32], F32R, tag="x5T")
        for kb in range(2):
            trF = psp.tile([128, 32], F32, tag="tr", name="trF", bufs=2)
            nc.tensor.transpose(trF[:], x5[:, 128 * kb:128 * (kb + 1)],
                                v32("identf", rows=32))
            nc.vector.tensor_copy(x5T[:, kb, :], trF[:])

        psG = psp.tile([32, 24], F32, tag="tr", name="psG", bufs=2)
        nc.tensor.matmul(psG[:], ones32[:],
                         vfr("fb2", rows=1),
                         start=True, stop=False)
        nc.tensor.matmul(psG[:], x5T[:, 0, :],
                         vfr("fw2")[:, 0, :],
                         start=False, stop=False)
        nc.tensor.matmul(psG[:], x5T[:, 1, :],
                         vfr("fw2")[:, 1, :],
                         start=False, stop=True)
        osb = smp.tile([32, 24], F32, tag="osb")
        nc.scalar.copy(osb[:, 0:2], psG[:, 0:2])
        nc.scalar.activation(osb[:, 2:4], psG[:, 2:4], ACTF.Sigmoid)
        nc.scalar.copy(osb[:, 4:24], psG[:, 4:24])
        nc.sync.dma_start(out=out_d, in_=osb[:])

    nc.compile()
    return nc


# ------------------------------------------------------------------ entry

_CACHE = {}


def _get_nc():
    if "nc" not in _CACHE:
        _CACHE["nc"] = build_nc()
    return _CACHE["nc"]


def make_in_maps(**inputs):
    consts = _build_consts(
        inputs["w1"], inputs["b1"], inputs["w2"], inputs["b2"],
        inputs["w3"], inputs["b3"], inputs["w4"], inputs["b4"],
        inputs["fw1"], inputs["fb1"], inputs["fw2"], inputs["fb2"],
    )
    image = np.ascontiguousarray(np.asarray(inputs["image"], np.float32))
    features = np.ascontiguousarray(np.asarray(inputs["features"], np.float32))
    centers = np.ascontiguousarray(np.asarray(inputs["centers"], np.float32))
    in_maps = []
    for i in range(NCORES):
        sl = slice(i * BL, (i + 1) * BL)
        m = {
            "image": np.ascontiguousarray(image[sl]),
            "features": np.ascontiguousarray(features[sl]),
            "centers": np.ascontiguousarray(centers[sl]),
        }
        m.update(consts)
        in_maps.append(m)
    return in_maps


def kernel(**inputs):
    nc = _get_nc()
    in_maps = make_in_maps(**inputs)
    res = run_bass_kernel_spmd(nc, in_maps, core_ids=list(range(NCORES)))
    out = np.concatenate([res.results[i]["out"] for i in range(NCORES)], axis=0)
    return out.astype(np.float32)
